# revision 12
# baseline (speedup 1.0000x reference)
"""Trainium2 Bass kernel for nn_Conv2d_61881888800824.

StyleGAN2-style synthesis layer:
    renorm(weight) -> upsample x2 (zero-insert) -> 4x4 FIR -> 3x3 conv
    -> + noise*strength -> + bias -> leaky_relu(0.2) * sqrt(2)

Math: the upsample+FIR+conv chain is folded (host-side) into a polyphase
decomposition — 4 independent 3x3 convolutions of the ORIGINAL 64x64 input
(one per output-pixel parity phase (a,b)).  Each phase conv is evaluated
with a 1-D horizontal Winograd F(4,3) decomposition: 6 transform points per
4 output columns, vertical taps kept direct, so the TensorEngine streams
6*3/(4*3) = 1.5x fewer MACs... concretely 4.5 mult/output vs 9 direct = 2x
fewer PE rows than the direct method.  Input transform V = B^T s is
computed once on DVE/GpSimd (shared by all 4 phases and both cout tiles);
the output transform A^T Y runs on DVE from SBUF copies of PSUM (copied by
the Activation engine, which also applies bias + leaky-relu).  Matmuls run
in fp16 (1 cycle/row on PE, fp32 PSUM accumulation).

Sharding: data-parallel over batch — 1 image per NeuronCore, 8 cores.

Self-contained: inputs are the full tensors from setup_inputs(); output is
the full [8, 256, 128, 128] fp32 array.
"""
from contextlib import ExitStack

import numpy as np

import bass_rust
import concourse.bass as bass
import concourse.mybir as mybir
import concourse.tile as tile
from concourse.bass_utils import run_bass_kernel_spmd

F32 = mybir.dt.float32
F16 = mybir.dt.float16
MULT = mybir.AluOpType.mult
ADD = mybir.AluOpType.add
MAX = mybir.AluOpType.max


# ---------------------------------------------------------------------------
# Wait legalization: this walrus build accepts at most ONE embedded sync wait
# per instruction. Tile can emit more (incl. same-engine self-waits that are
# provably satisfied by the engine's serial program order). Drop the provable
# ones; move the rest onto standalone EventSemaphore instructions inserted
# just before the over-limit instruction on the same engine.
# ---------------------------------------------------------------------------

def _is_async_update(inst) -> bool:
    n = type(inst).__name__
    return 'DMA' in n or 'Swdge' in n or 'Collective' in n or 'Dma' in n


def legalize_waits(nc, evsem_limit: int = 1) -> int:
    n_fixed = 0
    for fn in nc.m.functions:
        for bb in fn.blocks:
            insts = bb.instructions
            cum: dict[tuple, int] = {}
            out = []
            changed = False
            for inst in insts:
                si = inst.sync_info
                waits = list(si.on_wait) if si is not None and si.on_wait else []
                updates = list(si.on_update) if si is not None and si.on_update else []
                eng = inst.engine
                limit = 1
                if len(waits) > limit:
                    kept = []
                    for w in waits:
                        if (w.sync_type == 'semaphore'
                                and w.wait_mode == 'sem-ge-imm'
                                and w.wait_reg is None
                                and cum.get((eng, w.id), 0) >= w.wait_value):
                            continue
                        kept.append(w)
                    waits = kept
                if len(waits) > limit:
                    excess = waits[:-limit]
                    waits = waits[-limit:]
                    while excess:
                        take, excess = excess[:evsem_limit], excess[evsem_limit:]
                        ev = mybir.InstEventSemaphore(
                            name=nc.get_next_instruction_name(), ins=[], outs=[])
                        ev.engine = eng
                        ev.sync_info = bass_rust.SyncInfo(on_wait=take, on_update=[])
                        out.append(ev)
                    inst.sync_info = bass_rust.SyncInfo(on_wait=waits,
                                                        on_update=updates)
                    changed = True
                    n_fixed += 1
                elif si is not None and len(list(si.on_wait or [])) != len(waits):
                    inst.sync_info = bass_rust.SyncInfo(on_wait=waits,
                                                        on_update=updates)
                    changed = True
                    n_fixed += 1
                out.append(inst)
                if not _is_async_update(inst):
                    for u in updates:
                        if (u.sync_type == 'semaphore'
                                and u.update_mode == 'sem-inc'
                                and u.update_reg is None):
                            k = (eng, u.id)
                            cum[k] = cum.get(k, 0) + u.update_value
            if changed:
                bb.instructions = out
    return n_fixed


# ---------------------------------------------------------------------------
# Device kernel (per core: one batch image)
# ---------------------------------------------------------------------------

def build_conv_nc(legalize=True):
    nc = bass.Bass("TRN2", target_bir_lowering=False, debug=False)
    xin = nc.dram_tensor("xin", [512, 64, 64], F32, kind="ExternalInput").ap()
    # wq[a, b, ct, ci, (cg j ky co)] fp16 Winograd-transformed weights
    wq = nc.dram_tensor("wq", [2, 2, 2, 128, 4 * 6 * 3 * 128], F16,
                        kind="ExternalInput").ap()
    # noise (already * strength * sqrt2): [a, r, b, c] fp16
    noise4 = nc.dram_tensor("noise4", [2, 64, 2, 64], F16,
                            kind="ExternalInput").ap()
    biasq = nc.dram_tensor("biasq", [128, 2], F32, kind="ExternalInput").ap()
    y = nc.dram_tensor("y", [256, 128, 128], F32, kind="ExternalOutput").ap()

    with ExitStack() as ctx:
        tc = ctx.enter_context(tile.TileContext(nc))
        vp = ctx.enter_context(tc.tile_pool(name="v", bufs=4))
        wp = ctx.enter_context(tc.tile_pool(name="w", bufs=5))
        np_ = ctx.enter_context(tc.tile_pool(name="noise", bufs=2))
        bp = ctx.enter_context(tc.tile_pool(name="bias", bufs=1))
        pp = ctx.enter_context(tc.tile_pool(name="psum", bufs=8, space="PSUM"))

        bias_sb = bp.tile([128, 2], F32)
        nc.sync.dma_start(bias_sb[:], biasq)

        vt = [vp.tile([128, 66, 6, 16], F16, tag="v", name=f"v{i}")
              for i in range(4)]
        wslabs = {}
        nzt = {}

        # ------------------------------------------------------------------
        # Pass 1: input transform.  V[cg][ci, r, j, t] = sum_k BT[j,k] *
        # xpad[ci, r, 4t+k]  (xpad = 66x66 zero-bordered input), fp16 out.
        # B^T rows: [4,0,-5,0,1,0] [0,-4,-4,1,1,0] [0,4,-4,-1,1,0]
        #           [0,-2,-1,2,1,0] [0,2,-1,-2,1,0] [0,4,0,-5,0,1]
        # Scoped pools: released after pass 1, reused by the epilogue pools.
        # ------------------------------------------------------------------
        with tc.tile_pool(name="x", bufs=2) as xp, \
                tc.tile_pool(name="tmp", bufs=2) as tp, \
                tc.tile_pool(name="gtmp", bufs=4) as tg:
            xqs = []
            for cg in range(4):
                xq = xp.tile([128, 66, 66], F32, tag="x", name=f"x{cg}")
                xqs.append(xq)
                for sl in (xq[:, 0, :], xq[:, 65, :], xq[:, :, 0],
                           xq[:, :, 65]):
                    nc.gpsimd.memset(sl, 0.0)
                nc.sync.dma_start(xq[:, 1:33, 1:65],
                                  xin[cg * 128:(cg + 1) * 128, 0:32])
                nc.sync.dma_start(xq[:, 33:65, 1:65],
                                  xin[cg * 128:(cg + 1) * 128, 32:64])
                if cg == 0:
                    # first-group weights + a=0 noise right after cg0's x
                    for b in range(2):
                        for h in range(2):
                            t = wp.tile([128, 2, 6, 3, 128], F16, tag="wslab",
                                        name=f"w0{b}0{h}")
                            src = wq[0, b, 0].rearrange(
                                "ci (cg j ky co) -> ci cg j ky co",
                                cg=4, j=6, ky=3)
                            nc.sync.dma_start(t[:], src[:, 2 * h:2 * h + 2])
                            wslabs[(0, b, 0, h)] = t
                    nt = np_.tile([128, 64, 2, 64], F16, tag="nz", name="nz0")
                    nc.sync.dma_start(nt[:], bass.AP(
                        tensor=noise4.tensor, offset=0,
                        ap=[[0, 128], [128, 64], [64, 2], [1, 64]]))
                    nzt[0] = nt

                def s(k):
                    return xq[:, :, k:k + 61:4]

                V = vt[cg]
                # gpsimd: difference temps
                d1 = tg.tile([128, 66, 16], F32, tag="g", name=f"d1_{cg}")
                d2 = tg.tile([128, 66, 16], F32, tag="g", name=f"d2_{cg}")
                e1 = tg.tile([128, 66, 16], F32, tag="g", name=f"e1_{cg}")
                e2 = tg.tile([128, 66, 16], F32, tag="g", name=f"e2_{cg}")
                nc.gpsimd.tensor_sub(d1[:], s(1), s(2))
                nc.gpsimd.tensor_sub(d2[:], s(4), s(3))
                nc.gpsimd.tensor_sub(e1[:], s(3), s(1))
                nc.gpsimd.tensor_sub(e2[:], s(4), s(2))
                # DVE: sums + all six outputs (cast to fp16 on write)
                t0 = tp.tile([128, 66, 16], F32, tag="t", name=f"t0_{cg}")
                t1 = tp.tile([128, 66, 16], F32, tag="t", name=f"t1_{cg}")
                t2 = tp.tile([128, 66, 16], F32, tag="t", name=f"t2_{cg}")
                t3 = tp.tile([128, 66, 16], F32, tag="t", name=f"t3_{cg}")
                nc.vector.scalar_tensor_tensor(t0[:], s(0), 4.0, s(4),
                                               MULT, ADD)
                nc.vector.scalar_tensor_tensor(V[:, :, 0, :], s(2), -5.0,
                                               t0[:], MULT, ADD)
                nc.vector.tensor_add(t1[:], s(1), s(2))
                nc.vector.tensor_add(t2[:], s(3), s(4))
                nc.vector.scalar_tensor_tensor(V[:, :, 1, :], t1[:], -4.0,
                                               t2[:], MULT, ADD)
                nc.vector.scalar_tensor_tensor(V[:, :, 2, :], d1[:], 4.0,
                                               d2[:], MULT, ADD)
                nc.vector.scalar_tensor_tensor(V[:, :, 3, :], e1[:], 2.0,
                                               e2[:], MULT, ADD)
                nc.vector.scalar_tensor_tensor(V[:, :, 4, :], e1[:], -2.0,
                                               e2[:], MULT, ADD)
                nc.vector.scalar_tensor_tensor(t3[:], s(1), 4.0, s(5),
                                               MULT, ADD)
                nc.vector.scalar_tensor_tensor(V[:, :, 5, :], s(3), -5.0,
                                               t3[:], MULT, ADD)

        # ------------------------------------------------------------------
        # Main loop.  Output final y[co, 32ub+2u+a, 8t+2c+b].
        # A^T rows: [1,1,1,1,1,0] [0,1,-1,2,-2,0] [0,1,1,4,4,0] [0,1,-1,8,-8,1]
        # Both b-phases merged into one DVE col-pass over [128, 16, 2, 16].
        # ------------------------------------------------------------------
        with tc.tile_pool(name="stg", bufs=2) as sp, \
                tc.tile_pool(name="cpt", bufs=16) as cp, \
                tc.tile_pool(name="lrs", bufs=1) as lp:
            for a in range(2):
                if a not in nzt:
                    nt = np_.tile([128, 64, 2, 64], F16, tag="nz",
                                  name=f"nz{a}")
                    nc.sync.dma_start(nt[:], bass.AP(
                        tensor=noise4.tensor, offset=a * 8192,
                        ap=[[0, 128], [128, 64], [64, 2], [1, 64]]))
                    nzt[a] = nt
                nz = nzt[a]
                for ct in range(2):
                    if not (a == 0 and ct == 0):
                        for b in range(2):
                            for h in range(2):
                                t = wp.tile([128, 2, 6, 3, 128], F16,
                                            tag="wslab",
                                            name=f"w{a}{b}{ct}{h}")
                                src = wq[a, b, ct].rearrange(
                                    "ci (cg j ky co) -> ci cg j ky co",
                                    cg=4, j=6, ky=3)
                                nc.sync.dma_start(t[:],
                                                  src[:, 2 * h:2 * h + 2])
                                wslabs[(a, b, ct, h)] = t
                    for ub in range(4):
                        stg = sp.tile([128, 16, 128], F32, tag="stg")
                        yc = [cp.tile([128, 16, 2, 16], F32, tag="c",
                                      name=f"yc{a}{ct}{ub}_{j}")
                              for j in range(6)]
                        for b in range(2):
                            ps = [pp.tile([128, 16, 16], F32, tag="ps",
                                          name=f"ps{a}{ct}{ub}{b}_{j}")
                                  for j in range(6)]
                            for cg in range(4):
                                w = wslabs[(a, b, ct, cg // 2)]
                                for j in range(6):
                                    for ky in range(3):
                                        nc.tensor.matmul(
                                            ps[j][:],
                                            w[:, cg % 2, j, ky, :],
                                            vt[cg][:, 16 * ub + ky:
                                                   16 * ub + ky + 16, j, :],
                                            start=(cg == 0 and ky == 0),
                                            stop=(cg == 3 and ky == 2),
                                        )
                            # Act: drain PSUM -> SBUF (frees banks, feeds DVE)
                            for j in range(6):
                                nc.scalar.copy(yc[j][:, :, b, :], ps[j][:])
                        # DVE: A^T column transform; combine stage merged
                        # over both b (contiguous => 2D canonical), final
                        # bias+noise writes per-b (3D strided views).
                        S = cp.tile([128, 16, 2, 16], F32, tag="c")
                        D = cp.tile([128, 16, 2, 16], F32, tag="c")
                        S2 = cp.tile([128, 16, 2, 16], F32, tag="c")
                        D2 = cp.tile([128, 16, 2, 16], F32, tag="c")
                        w0 = cp.tile([128, 16, 2, 16], F32, tag="c")
                        w0b = cp.tile([128, 16, 2, 16], F32, tag="c")
                        w1 = cp.tile([128, 16, 2, 16], F32, tag="c")
                        w2 = cp.tile([128, 16, 2, 16], F32, tag="c")
                        w3 = cp.tile([128, 16, 2, 16], F32, tag="c")
                        w3b = cp.tile([128, 16, 2, 16], F32, tag="c")
                        nc.vector.tensor_add(S[:], yc[1][:], yc[2][:])
                        nc.vector.tensor_sub(D[:], yc[1][:], yc[2][:])
                        nc.vector.tensor_add(S2[:], yc[3][:], yc[4][:])
                        nc.vector.tensor_sub(D2[:], yc[3][:], yc[4][:])
                        nc.vector.tensor_add(w0[:], yc[0][:], S[:])
                        nc.vector.tensor_add(w0b[:], w0[:], S2[:])
                        nc.vector.scalar_tensor_tensor(w1[:], D2[:], 2.0,
                                                       D[:], MULT, ADD)
                        nc.vector.scalar_tensor_tensor(w2[:], S2[:], 4.0,
                                                       S[:], MULT, ADD)
                        nc.vector.scalar_tensor_tensor(w3[:], D2[:], 8.0,
                                                       D[:], MULT, ADD)
                        nc.vector.tensor_add(w3b[:], w3[:], yc[5][:])
                        pre = [w0b, w1, w2, w3b]
                        bias_ap = bias_sb[:, ct:ct + 1]
                        for b in range(2):
                            for c in range(4):
                                nc.vector.scalar_tensor_tensor(
                                    stg[:, :, 2 * c + b::8],
                                    pre[c][:, :, b, :], bias_ap,
                                    nz[:, 16 * ub:16 * ub + 16, b,
                                       c:c + 61:4],
                                    ADD, ADD)
                        # leaky relu: relu(0.8x) + 0.2x (Pool TT has no
                        # max op on core v3; Act computes both scaled parts)
                        ls1 = lp.tile([128, 16, 128], F32, tag="ls1")
                        ls2 = lp.tile([128, 16, 128], F32, tag="ls2")
                        nc.scalar.activation(
                            ls1[:], stg[:],
                            mybir.ActivationFunctionType.Relu,
                            bias=0.0, scale=0.8)
                        nc.scalar.mul(ls2[:], stg[:], 0.2)
                        nc.gpsimd.tensor_add(stg[:], ls1[:], ls2[:])
                        ydst = bass.AP(
                            tensor=y.tensor,
                            offset=(ct * 128) * 16384 + (32 * ub + a) * 128,
                            ap=[[16384, 128], [256, 16], [1, 128]],
                        )
                        nc.sync.dma_start(ydst, stg[:])
    if legalize:
        legalize_waits(nc)
    return nc


# ---------------------------------------------------------------------------
# Host-side preparation (renorm + FIR folding + phase + Winograd transform)
# ---------------------------------------------------------------------------

def prep_inputs(x, weight, bias, noise_const, noise_strength):
    SQ2 = np.sqrt(2.0)
    G = np.array([[1 / 4, 0, 0], [-1 / 6, -1 / 6, -1 / 6],
                  [-1 / 6, 1 / 6, -1 / 6], [1 / 24, 1 / 12, 1 / 6],
                  [1 / 24, -1 / 12, 1 / 6], [0, 0, 1]], dtype=np.float64)
    w = np.asarray(weight).astype(np.float64)
    inv = 1.0 / np.sqrt((w ** 2).sum(axis=(1, 2, 3)) + 1e-8)
    w = w * inv[:, None, None, None]
    f = np.outer([1., 3., 3., 1.], [1., 3., 3., 1.])
    f = f / f.sum() * 4.0                       # FIR * up^2 gain
    wf = w[:, :, ::-1, ::-1]                    # flipped (cross-corr of flip)
    g = np.zeros((w.shape[0], w.shape[1], 6, 6))
    for m in range(3):
        for n in range(3):
            g[:, :, m:m + 4, n:n + 4] += wf[:, :, m, n, None, None] * f

    # wq[a, b, ct, ci, (cg j ky co)] = G @ h_ab (horizontal Winograd)
    wq = np.empty((2, 2, 2, 128, 4 * 6 * 3 * 128), dtype=np.float16)
    for a in range(2):
        for b in range(2):
            h = g[:, :, (1 - a)::2, (1 - b)::2] * SQ2    # [co, ci, ky, kx]
            Wp = np.einsum('jk,oiyk->jyio', G, h)        # [j, ky, ci, co]
            arr = Wp.reshape(6, 3, 4, 128, 2, 128)       # j ky cg ci ct co
            arr = arr.transpose(4, 3, 2, 0, 1, 5)        # ct ci cg j ky co
            wq[a, b] = arr.reshape(2, 128, 4 * 6 * 3 * 128).astype(np.float16)

    noise2 = (np.asarray(noise_const).astype(np.float64)
              * float(noise_strength) * SQ2)
    noise4 = np.empty((2, 64, 2, 64), dtype=np.float16)
    for a in range(2):
        for b in range(2):
            noise4[a, :, b, :] = noise2[a::2, b::2].astype(np.float16)

    biasq = np.empty((128, 2), dtype=np.float32)
    bias2 = np.asarray(bias).astype(np.float64) * SQ2
    biasq[:, 0] = bias2[:128]
    biasq[:, 1] = bias2[128:]

    x = np.asarray(x)
    return [{
        "xin": np.ascontiguousarray(x[bi], dtype=np.float32),
        "wq": wq,
        "noise4": noise4,
        "biasq": biasq,
    } for bi in range(x.shape[0])]


_NC_CACHE = None


def kernel(x, weight, bias, noise_const, noise_strength):
    global _NC_CACHE
    if _NC_CACHE is None:
        _NC_CACHE = build_conv_nc()
    in_maps = prep_inputs(x, weight, bias, noise_const, noise_strength)
    res = run_bass_kernel_spmd(_NC_CACHE, in_maps, core_ids=list(range(8)))
    return np.ascontiguousarray(
        np.stack([r["y"] for r in res.results]), dtype=np.float32)
